# revision 6
# baseline (speedup 1.0000x reference)
"""BiDAF attention Bass kernel for Trainium2 (8 NeuronCores, batch-parallel).

Takes FULL inputs (BS=32, MCL=1024, MQL=64, d=512), shards batch across the
8 cores (4 batches/core), runs one SPMD Bass kernel, gathers the full output
(32, 1024, 2048) float32.

Self-contained: only imports concourse (available on sys.path in the
container via sitecustomize).
"""
import sys

if "/opt/trn_rl_repo" not in sys.path:
    sys.path.insert(0, "/opt/trn_rl_repo")

from contextlib import ExitStack

import numpy as np

import concourse.bass as bass
import concourse.bacc as bacc
import concourse.tile as tile
from concourse import mybir
from concourse.masks import make_identity

dt = mybir.dt
AF = mybir.ActivationFunctionType
ALU = mybir.AluOpType
AX = mybir.AxisListType

NCORES = 8
BS, MCL, MQL, D = 32, 1024, 64, 512
BPC = BS // NCORES          # batches per core
NT = MCL // 128             # c-tiles per batch
NK = D // 128               # contraction chunks
F32 = dt.float32
MM = dt.float32r            # matmul dtype (fast fp32 path on PE)
EXP_BIAS = -3.0             # constant shift inside c2q softmax (exact in softmax math)


def build_nc():
    nc = bacc.Bacc("TRN2", target_bir_lowering=False)
    hq_d = nc.dram_tensor("hq", [BPC, MQL, D], F32, kind="ExternalInput")
    hc_d = nc.dram_tensor("hc", [BPC, MCL, D], F32, kind="ExternalInput")
    w_d = nc.dram_tensor("w", [3 * D, 1], F32, kind="ExternalInput")
    out_d = nc.dram_tensor("out", [BPC, MCL, 4 * D], F32, kind="ExternalOutput")

    with tile.TileContext(nc) as tc, ExitStack() as ctx:
        const = ctx.enter_context(tc.tile_pool(name="const", bufs=1))
        sb = ctx.enter_context(tc.tile_pool(name="sb", bufs=2))
        ps1 = ctx.enter_context(tc.tile_pool(name="ps1", bufs=1, space="PSUM"))
        ps2 = ctx.enter_context(tc.tile_pool(name="ps2", bufs=2, space="PSUM"))

        # ---- constants ----
        ident = const.tile([128, 128], F32)
        make_identity(nc, ident[:])
        identR = const.tile([128, 128], MM)
        nc.vector.tensor_copy(identR[:], ident[:])
        ones32 = const.tile([128, 1], F32)
        nc.vector.memset(ones32[:], 1.0)
        ones32r = const.tile([1, 512], F32)
        nc.vector.memset(ones32r[:], 1.0)
        ones_r = const.tile([1, 512], MM)
        nc.vector.tensor_copy(ones_r[:], ones32r[:])
        # W as (128, 12): col j holds W[j*128 : (j+1)*128]; j=0..3 Wc, 4..7 Wq, 8..11 Wm
        wv = const.tile([128, 12], F32)
        nc.sync.dma_start(wv[:], w_d.rearrange("(j p) o -> p (j o)", p=128))
        wvr = const.tile([128, 12], MM)
        nc.vector.tensor_copy(wvr[:], wv[:])
        bias_e = const.tile([128, 1], F32)
        nc.vector.memset(bias_e[:], EXP_BIAS)
        bias_0 = const.tile([128, 1], F32)
        nc.vector.memset(bias_0[:], 0.0)

        for b in range(BPC):
            # ---- loads (inputs on the SP queue; outputs go on gpsimd) ----
            hq_r = sb.tile([MQL, D], MM, tag="hq")
            nc.sync.dma_start(hq_r[:], hq_d[b].bitcast(MM))
            hc_nat = sb.tile([128, NT, D], MM, tag="hc")
            hc_src = hc_d[b].rearrange("(t p) d -> p t d", p=128).bitcast(MM)
            for hf in range(2):
                nc.sync.dma_start(hc_nat[:, hf * 4:(hf + 1) * 4, :],
                                  hc_src[:, hf * 4:(hf + 1) * 4, :])

            # ---- Hq^T (d on partitions) ----
            hqT_ps = ps1.tile([128, NK, MQL], MM, tag="sbps")
            for k in range(NK):
                nc.tensor.transpose(
                    hqT_ps[:, k, :], hq_r[:, k * 128:(k + 1) * 128],
                    identR[0:MQL, 0:MQL])
            hqT_s = sb.tile([128, NK, MQL], MM, tag="hqT")
            nc.vector.tensor_copy(hqT_s[:], hqT_ps[:])

            # ---- stationary weights for S^T: [HqT*Wm | Wc] per chunk ----
            stw = sb.tile([128, NK, MQL + 1], MM, tag="stw")
            for k in range(NK):
                nc.vector.tensor_scalar(
                    stw[:, k, 0:MQL], hqT_s[:, k, :].bitcast(F32),
                    wv[:, 8 + k, None], None, op0=ALU.mult)
                nc.vector.tensor_copy(stw[:, k, MQL:MQL + 1], wvr[:, k:k + 1])

            # ---- sq = Hq @ Wq  -> (1, 64); den2 shares the bank (col 64) ----
            sqden_ps = ps1.tile([1, MQL + 1], F32, tag="small")
            for k in range(NK):
                nc.tensor.matmul(sqden_ps[0:1, 0:MQL], wvr[:, 4 + k, None],
                                 hqT_s[:, k, :],
                                 start=(k == 0), stop=(k == NK - 1))
            aug32 = sb.tile([1, MQL + 1], F32, tag="aug32")
            nc.vector.memset(aug32[:], 0.0)
            nc.vector.tensor_copy(aug32[0:1, 0:MQL], sqden_ps[0:1, 0:MQL])
            aug = sb.tile([1, MQL + 1], MM, tag="aug")
            nc.vector.tensor_copy(aug[:], aug32[:])

            # ---- Hc^T via PE transposes ----
            hcT = sb.tile([128, NK, MCL], MM, tag="hcT")
            for g in range(8):
                k, hf = divmod(g, 2)
                trp = ps2.tile([128, 512], MM, tag="trp")
                for j in range(4):
                    t = hf * 4 + j
                    nc.tensor.transpose(
                        trp[:, j * 128:(j + 1) * 128],
                        hc_nat[:, t, k * 128:(k + 1) * 128], identR[:])
                nc.any.tensor_copy(hcT[:, k, hf * 512:(hf + 1) * 512], trp[:])

            # ---- S^T = [T | sc]^T : (65, 1024) ----
            sT_s = sb.tile([MQL + 1, MCL], F32, tag="sT")
            for hf in range(2):
                sT_ps = ps2.tile([MQL + 1, 512], F32, tag="swps")
                for k in range(NK):
                    nc.tensor.matmul(
                        sT_ps[:], stw[:, k, :],
                        hcT[:, k, hf * 512:(hf + 1) * 512],
                        start=(k == 0), stop=False)
                nc.tensor.matmul(sT_ps[:], aug[:], ones_r[0:1, 0:512],
                                 start=False, stop=True)
                nc.vector.tensor_copy(sT_s[:, hf * 512:(hf + 1) * 512], sT_ps[:])

            # ---- S tiles (128, 65) + softmax stats ----
            score = sb.tile([128, NT], F32, tag="score")
            nm = sb.tile([128, NT], F32, tag="nm")
            dens = sb.tile([128, NT], F32, tag="dens")
            rec = sb.tile([128, NT], F32, tag="rec")
            E = sb.tile([128, NT, MQL], F32, tag="E")
            for j in range(2):
                sbank = ps1.tile([128, 4, MQL + 1], F32, tag="sbps")
                for i in range(4):
                    t = j * 4 + i
                    nc.tensor.transpose(
                        sbank[:, i, :], sT_s[:, t * 128:(t + 1) * 128],
                        ident[0:MQL + 1, 0:MQL + 1])
                nc.vector.tensor_reduce(
                    nm[:, j * 4:(j + 1) * 4], sbank[:, :, 0:MQL],
                    axis=AX.X, op=ALU.max, negate=True)
                nc.scalar.activation(
                    E[:, j * 4:(j + 1) * 4, :], sbank[:, :, 0:MQL],
                    AF.Exp, bias=bias_e[:], scale=1.0)
                nc.vector.tensor_reduce(
                    dens[:, j * 4:(j + 1) * 4], E[:, j * 4:(j + 1) * 4, :],
                    axis=AX.X, op=ALU.add)
                # score = sc + rowmax = sc - nm
                nc.vector.tensor_tensor(
                    score[:, j * 4:(j + 1) * 4], sbank[:, :, MQL],
                    nm[:, j * 4:(j + 1) * 4], op=ALU.subtract)
            nc.vector.reciprocal(rec[:], dens[:])

            # ---- q2c attention: e2 = exp(score), qac ----
            e2 = sb.tile([128, NT], F32, tag="e2")
            nc.scalar.activation(e2[:], score[:], AF.Exp, bias=bias_0[:], scale=1.0)
            e2r = sb.tile([128, NT], MM, tag="e2r")
            nc.vector.tensor_copy(e2r[:], e2[:])
            dsum = sb.tile([128, 1], F32, tag="dsum")
            nc.vector.tensor_reduce(dsum[:], e2[:], axis=AX.X, op=ALU.add)
            nc.tensor.matmul(sqden_ps[0:1, MQL:MQL + 1], dsum[:], ones32[:],
                             start=True, stop=True)
            rec2 = sb.tile([1, 1], F32, tag="rec2")
            nc.vector.reciprocal(rec2[:], sqden_ps[0:1, MQL:MQL + 1])
            U_ps = ps1.tile([1, D], F32, tag="small")
            for t in range(NT):
                nc.tensor.matmul(U_ps[:], e2r[:, t:t + 1], hc_nat[:, t, :],
                                 start=(t == 0), stop=(t == NT - 1))
            qacT = sb.tile([1, D], F32, tag="qacT")
            nc.vector.tensor_scalar(qacT[:], U_ps[:], rec2[:], None, op0=ALU.mult)
            qacB = sb.tile([128, D], F32, tag="qacB")
            nc.gpsimd.partition_broadcast(qacB[:], qacT[:])

            # ---- c2q weight transposes: wT = E^T ----
            wT = sb.tile([MQL, NT, 128], MM, tag="wT")
            for j in range(2):
                wT_ps = ps2.tile([MQL, 512], F32, tag="swps")
                for i in range(4):
                    t = j * 4 + i
                    nc.tensor.transpose(
                        wT_ps[:, i * 128:(i + 1) * 128], E[:, t, :], ident[:])
                nc.vector.tensor_copy(wT[:, j * 4:(j + 1) * 4, :], wT_ps[:])

            # ---- A = w @ Hq, output assembly ----
            out_view = out_d[b].rearrange("(t p) j -> p t j", p=128)
            for hf in range(2):
                out_t = sb.tile([128, 4, 4 * D], F32, tag="out")
                tmp = sb.tile([128, 4, 2 * D], F32, tag="tmp")
                for i in range(4):
                    t = hf * 4 + i
                    A_ps = ps2.tile([128, D], F32, tag="A")
                    nc.tensor.matmul(A_ps[:], wT[:, t, :], hq_r[:],
                                     start=True, stop=True)
                    nc.scalar.activation(out_t[:, i, D:2 * D], A_ps[:],
                                         AF.Tanh, bias=bias_0[:],
                                         scale=rec[:, t:t + 1])
                    nc.vector.scalar_tensor_tensor(
                        tmp[:, i, 0:D], A_ps[:], rec[:, t:t + 1],
                        hc_nat[:, t, :].bitcast(F32), op0=ALU.mult, op1=ALU.mult)
                    nc.vector.tensor_tensor(
                        tmp[:, i, D:2 * D], hc_nat[:, t, :].bitcast(F32),
                        qacB[:], op=ALU.mult)
                nc.scalar.activation(
                    out_t[:, :, 0:D],
                    hc_nat[:, hf * 4:(hf + 1) * 4, :].bitcast(F32),
                    AF.Tanh, bias=bias_0[:], scale=1.0)
                nc.scalar.activation(out_t[:, :, 2 * D:4 * D], tmp[:],
                                     AF.Tanh, bias=bias_0[:], scale=1.0)
                nc.gpsimd.dma_start(out_view[:, hf * 4:(hf + 1) * 4, :], out_t[:])
    nc.compile()
    return nc


_NC = None


def _get_nc():
    global _NC
    if _NC is None:
        _NC = build_nc()
    return _NC


def run(inputs: dict, trace: bool = False, tmpdir: str | None = None):
    """Shard, run on 8 cores, gather. Returns (out, BassKernelResults)."""
    from concourse.bass_utils import run_bass_kernel_spmd

    if trace:
        # the axon NTFF hook module is absent in this image; inject it
        try:
            from antenv import axon_hooks  # noqa: F401
        except ImportError:
            import types
            import antenv
            from trn_agent_boot.trn_boot import _ntff_profile_via_ctypes
            mod = types.ModuleType("antenv.axon_hooks")
            _hook = _ntff_profile_via_ctypes('/opt/axon/libaxon_pjrt.so')
            mod.get_axon_ntff_profile_hook = lambda: _hook
            mod.set_axon_ntff_profile_hook = lambda h: None
            sys.modules["antenv.axon_hooks"] = mod
            antenv.axon_hooks = mod

    Hq = np.ascontiguousarray(np.asarray(inputs["Hq"], dtype=np.float32))
    Hc = np.ascontiguousarray(np.asarray(inputs["Hc"], dtype=np.float32))
    W = np.ascontiguousarray(np.asarray(inputs["W"], dtype=np.float32))
    nc = _get_nc()
    in_maps = [
        {"hq": Hq[i * BPC:(i + 1) * BPC], "hc": Hc[i * BPC:(i + 1) * BPC], "w": W}
        for i in range(NCORES)
    ]
    br = run_bass_kernel_spmd(nc, in_maps, list(range(NCORES)), trace=trace,
                              tmpdir=tmpdir)
    out = np.concatenate([br.results[i]["out"] for i in range(NCORES)], axis=0)
    return out, br


def kernel(**inputs) -> np.ndarray:
    out, _ = run(inputs, trace=False)
    return out


# revision 7
# speedup vs baseline: 1.2380x; 1.2380x over previous
"""BiDAF attention Bass kernel for Trainium2 (8 NeuronCores, batch-parallel).

Takes FULL inputs (BS=32, MCL=1024, MQL=64, d=512), shards batch across the
8 cores (4 batches/core), runs one SPMD Bass kernel, gathers the full output
(32, 1024, 2048) float32.

Self-contained: only imports concourse (available on sys.path in the
container via sitecustomize).
"""
import sys

if "/opt/trn_rl_repo" not in sys.path:
    sys.path.insert(0, "/opt/trn_rl_repo")

from contextlib import ExitStack

import numpy as np

import concourse.bass as bass
import concourse.bacc as bacc
import concourse.tile as tile
from concourse import mybir
from concourse.masks import make_identity

dt = mybir.dt
AF = mybir.ActivationFunctionType
ALU = mybir.AluOpType
AX = mybir.AxisListType

NCORES = 8
BS, MCL, MQL, D = 32, 1024, 64, 512
BPC = BS // NCORES          # batches per core
NT = MCL // 128             # c-tiles per batch
NK = D // 128               # contraction chunks
F32 = dt.float32
MM = dt.float32r            # matmul dtype (fast fp32 path on PE)
EXP_BIAS = -3.0             # constant shift inside c2q softmax (exact in softmax math)


def build_nc():
    nc = bacc.Bacc("TRN2", target_bir_lowering=False)
    hq_d = nc.dram_tensor("hq", [BPC, MQL, D], F32, kind="ExternalInput")
    hc_d = nc.dram_tensor("hc", [BPC, MCL, D], F32, kind="ExternalInput")
    w_d = nc.dram_tensor("w", [3 * D, 1], F32, kind="ExternalInput")
    out_d = nc.dram_tensor("out", [BPC, MCL, 4 * D], F32, kind="ExternalOutput")

    with tile.TileContext(nc) as tc, ExitStack() as ctx:
        const = ctx.enter_context(tc.tile_pool(name="const", bufs=1))
        sb = ctx.enter_context(tc.tile_pool(name="sb", bufs=2))
        hcp = ctx.enter_context(tc.tile_pool(name="hcp", bufs=3))
        ps1 = ctx.enter_context(tc.tile_pool(name="ps1", bufs=1, space="PSUM"))
        ps2 = ctx.enter_context(tc.tile_pool(name="ps2", bufs=2, space="PSUM"))

        # ---- constants ----
        ident = const.tile([128, 128], F32)
        make_identity(nc, ident[:])
        identR = const.tile([128, 128], MM)
        nc.vector.tensor_copy(identR[:], ident[:])
        ones32 = const.tile([128, 1], F32)
        nc.vector.memset(ones32[:], 1.0)
        ones32r = const.tile([1, 512], F32)
        nc.vector.memset(ones32r[:], 1.0)
        ones_r = const.tile([1, 512], MM)
        nc.vector.tensor_copy(ones_r[:], ones32r[:])
        # W as (128, 12): col j holds W[j*128 : (j+1)*128]; j=0..3 Wc, 4..7 Wq, 8..11 Wm
        wv = const.tile([128, 12], F32)
        nc.sync.dma_start(wv[:], w_d.rearrange("(j p) o -> p (j o)", p=128))
        wvr = const.tile([128, 12], MM)
        nc.vector.tensor_copy(wvr[:], wv[:])
        bias_e = const.tile([128, 1], F32)
        nc.vector.memset(bias_e[:], EXP_BIAS)
        bias_0 = const.tile([128, 1], F32)
        nc.vector.memset(bias_0[:], 0.0)

        st = [dict() for _ in range(BPC)]   # per-batch live tiles

        def stage1(b):
            v = st[b]
            # ---- loads (inputs on the SP queue; outputs go on gpsimd) ----
            hq_r = sb.tile([MQL, D], MM, tag="hq")
            nc.sync.dma_start(hq_r[:], hq_d[b].bitcast(MM))
            hc_nat = hcp.tile([128, NT, D], MM, tag="hc")
            hc_src = hc_d[b].rearrange("(t p) d -> p t d", p=128).bitcast(MM)
            for hf in range(2):
                nc.sync.dma_start(hc_nat[:, hf * 4:(hf + 1) * 4, :],
                                  hc_src[:, hf * 4:(hf + 1) * 4, :])
            v["hq_r"], v["hc_nat"] = hq_r, hc_nat

            # ---- Hq^T (d on partitions) ----
            hqT_ps = ps1.tile([128, NK, MQL], MM, tag="sbps")
            for k in range(NK):
                nc.tensor.transpose(
                    hqT_ps[:, k, :], hq_r[:, k * 128:(k + 1) * 128],
                    identR[0:MQL, 0:MQL])
            hqT_s = sb.tile([128, NK, MQL], MM, tag="hqT")
            nc.vector.tensor_copy(hqT_s[:], hqT_ps[:])

            # ---- stationary weights for S^T: [HqT*Wm | Wc] per chunk ----
            stw = sb.tile([128, NK, MQL + 1], MM, tag="stw")
            for k in range(NK):
                nc.vector.tensor_scalar(
                    stw[:, k, 0:MQL], hqT_s[:, k, :].bitcast(F32),
                    wv[:, 8 + k, None], None, op0=ALU.mult)
                nc.vector.tensor_copy(stw[:, k, MQL:MQL + 1], wvr[:, k:k + 1])

            # ---- sq = Hq @ Wq  -> (1, 64); den2 shares the bank (col 64) ----
            sqden_ps = ps1.tile([1, MQL + 1], F32, tag="small")
            for k in range(NK):
                nc.tensor.matmul(sqden_ps[0:1, 0:MQL], wvr[:, 4 + k, None],
                                 hqT_s[:, k, :],
                                 start=(k == 0), stop=(k == NK - 1))
            aug32 = sb.tile([1, MQL + 1], F32, tag="aug32")
            nc.vector.memset(aug32[:], 0.0)
            nc.vector.tensor_copy(aug32[0:1, 0:MQL], sqden_ps[0:1, 0:MQL])
            aug = sb.tile([1, MQL + 1], MM, tag="aug")
            nc.vector.tensor_copy(aug[:], aug32[:])

            # ---- Hc^T via PE transposes ----
            hcT = sb.tile([128, NK, MCL], MM, tag="hcT")
            for g in range(8):
                k, hf = divmod(g, 2)
                trp = ps2.tile([128, 512], MM, tag="trp")
                for j in range(4):
                    t = hf * 4 + j
                    nc.tensor.transpose(
                        trp[:, j * 128:(j + 1) * 128],
                        hc_nat[:, t, k * 128:(k + 1) * 128], identR[:])
                nc.any.tensor_copy(hcT[:, k, hf * 512:(hf + 1) * 512], trp[:])

            # ---- S^T = [T | sc]^T : (65, 1024) ----
            sT_s = sb.tile([MQL + 1, MCL], F32, tag="sT")
            for hf in range(2):
                sT_ps = ps2.tile([MQL + 1, 512], F32, tag="swps")
                for k in range(NK):
                    nc.tensor.matmul(
                        sT_ps[:], stw[:, k, :],
                        hcT[:, k, hf * 512:(hf + 1) * 512],
                        start=(k == 0), stop=False)
                nc.tensor.matmul(sT_ps[:], aug[:], ones_r[0:1, 0:512],
                                 start=False, stop=True)
                nc.vector.tensor_copy(sT_s[:, hf * 512:(hf + 1) * 512], sT_ps[:])

            # ---- S tiles (128, 65) + softmax stats ----
            score = sb.tile([128, NT], F32, tag="score")
            nm = sb.tile([128, NT], F32, tag="nm")
            dens = sb.tile([128, NT], F32, tag="dens")
            rec = sb.tile([128, NT], F32, tag="rec")
            E = sb.tile([128, NT, MQL], F32, tag="E")
            for j in range(2):
                sbank = ps1.tile([128, 4, MQL + 1], F32, tag="sbps")
                for i in range(4):
                    t = j * 4 + i
                    nc.tensor.transpose(
                        sbank[:, i, :], sT_s[:, t * 128:(t + 1) * 128],
                        ident[0:MQL + 1, 0:MQL + 1])
                nc.vector.tensor_reduce(
                    nm[:, j * 4:(j + 1) * 4], sbank[:, :, 0:MQL],
                    axis=AX.X, op=ALU.max, negate=True)
                nc.scalar.activation(
                    E[:, j * 4:(j + 1) * 4, :], sbank[:, :, 0:MQL],
                    AF.Exp, bias=bias_e[:], scale=1.0)
                nc.vector.tensor_reduce(
                    dens[:, j * 4:(j + 1) * 4], E[:, j * 4:(j + 1) * 4, :],
                    axis=AX.X, op=ALU.add)
                # score = sc + rowmax = sc - nm
                nc.vector.tensor_tensor(
                    score[:, j * 4:(j + 1) * 4], sbank[:, :, MQL],
                    nm[:, j * 4:(j + 1) * 4], op=ALU.subtract)
            nc.vector.reciprocal(rec[:], dens[:])
            v["rec"] = rec

            # ---- q2c attention: e2 = exp(score), qac ----
            e2 = sb.tile([128, NT], F32, tag="e2")
            nc.scalar.activation(e2[:], score[:], AF.Exp, bias=bias_0[:], scale=1.0)
            e2r = sb.tile([128, NT], MM, tag="e2r")
            nc.vector.tensor_copy(e2r[:], e2[:])
            dsum = sb.tile([128, 1], F32, tag="dsum")
            nc.vector.tensor_reduce(dsum[:], e2[:], axis=AX.X, op=ALU.add)
            nc.tensor.matmul(sqden_ps[0:1, MQL:MQL + 1], dsum[:], ones32[:],
                             start=True, stop=True)
            rec2 = sb.tile([1, 1], F32, tag="rec2")
            nc.vector.reciprocal(rec2[:], sqden_ps[0:1, MQL:MQL + 1])
            U_ps = ps1.tile([1, D], F32, tag="small")
            for t in range(NT):
                nc.tensor.matmul(U_ps[:], e2r[:, t:t + 1], hc_nat[:, t, :],
                                 start=(t == 0), stop=(t == NT - 1))
            qacT = sb.tile([1, D], F32, tag="qacT")
            nc.vector.tensor_scalar(qacT[:], U_ps[:], rec2[:], None, op0=ALU.mult)
            qacB = sb.tile([128, D], F32, tag="qacB")
            nc.gpsimd.partition_broadcast(qacB[:], qacT[:])
            v["qacB"] = qacB

            # ---- c2q weight transposes: wT = E^T ----
            wT = sb.tile([MQL, NT, 128], MM, tag="wT")
            for j in range(2):
                wT_ps = ps2.tile([MQL, 512], F32, tag="swps")
                for i in range(4):
                    t = j * 4 + i
                    nc.tensor.transpose(
                        wT_ps[:, i * 128:(i + 1) * 128], E[:, t, :], ident[:])
                nc.vector.tensor_copy(wT[:, j * 4:(j + 1) * 4, :], wT_ps[:])
            v["wT"] = wT

        def stage2(b):
            v = st[b]
            hq_r, hc_nat, rec, qacB, wT = (v["hq_r"], v["hc_nat"], v["rec"],
                                           v["qacB"], v["wT"])
            out_view = out_d[b].rearrange("(t p) j -> p t j", p=128)
            for q in range(4):          # quarter = 2 c-tiles
                out_t = sb.tile([128, 2, 4 * D], F32, tag="out")
                tmp = sb.tile([128, 2, 2 * D], F32, tag="tmp")
                for i in range(2):
                    t = q * 2 + i
                    A_ps = ps2.tile([128, D], F32, tag="A")
                    nc.tensor.matmul(A_ps[:], wT[:, t, :], hq_r[:],
                                     start=True, stop=True)
                    nc.scalar.activation(out_t[:, i, D:2 * D], A_ps[:],
                                         AF.Tanh, bias=bias_0[:],
                                         scale=rec[:, t:t + 1])
                    nc.vector.scalar_tensor_tensor(
                        tmp[:, i, 0:D], A_ps[:], rec[:, t:t + 1],
                        hc_nat[:, t, :].bitcast(F32), op0=ALU.mult, op1=ALU.mult)
                    nc.vector.tensor_tensor(
                        tmp[:, i, D:2 * D], hc_nat[:, t, :].bitcast(F32),
                        qacB[:], op=ALU.mult)
                nc.scalar.activation(
                    out_t[:, :, 0:D],
                    hc_nat[:, q * 2:(q + 1) * 2, :].bitcast(F32),
                    AF.Tanh, bias=bias_0[:], scale=1.0)
                nc.scalar.activation(out_t[:, :, 2 * D:4 * D], tmp[:],
                                     AF.Tanh, bias=bias_0[:], scale=1.0)
                nc.gpsimd.dma_start(out_view[:, q * 2:(q + 1) * 2, :], out_t[:])

        # 2-stage software pipeline across batches
        stage1(0)
        stage1(1)
        stage2(0)
        stage1(2)
        stage2(1)
        stage1(3)
        stage2(2)
        stage2(3)
    nc.compile()
    return nc


_NC = None


def _get_nc():
    global _NC
    if _NC is None:
        _NC = build_nc()
    return _NC


def run(inputs: dict, trace: bool = False, tmpdir: str | None = None):
    """Shard, run on 8 cores, gather. Returns (out, BassKernelResults)."""
    from concourse.bass_utils import run_bass_kernel_spmd

    if trace:
        # the axon NTFF hook module is absent in this image; inject it
        try:
            from antenv import axon_hooks  # noqa: F401
        except ImportError:
            import types
            import antenv
            from trn_agent_boot.trn_boot import _ntff_profile_via_ctypes
            mod = types.ModuleType("antenv.axon_hooks")
            _hook = _ntff_profile_via_ctypes('/opt/axon/libaxon_pjrt.so')
            mod.get_axon_ntff_profile_hook = lambda: _hook
            mod.set_axon_ntff_profile_hook = lambda h: None
            sys.modules["antenv.axon_hooks"] = mod
            antenv.axon_hooks = mod

    Hq = np.ascontiguousarray(np.asarray(inputs["Hq"], dtype=np.float32))
    Hc = np.ascontiguousarray(np.asarray(inputs["Hc"], dtype=np.float32))
    W = np.ascontiguousarray(np.asarray(inputs["W"], dtype=np.float32))
    nc = _get_nc()
    in_maps = [
        {"hq": Hq[i * BPC:(i + 1) * BPC], "hc": Hc[i * BPC:(i + 1) * BPC], "w": W}
        for i in range(NCORES)
    ]
    br = run_bass_kernel_spmd(nc, in_maps, list(range(NCORES)), trace=trace,
                              tmpdir=tmpdir)
    out = np.concatenate([br.results[i]["out"] for i in range(NCORES)], axis=0)
    return out, br


def kernel(**inputs) -> np.ndarray:
    out, _ = run(inputs, trace=False)
    return out
